# revision 12
# baseline (speedup 1.0000x reference)
# Self-contained Trainium2 Bass kernel for nn_MultiInputLSTMCell.
#
# Reference computation (all fp32):
#   pre   = h0 @ W_hh + bias + input_ @ W_ih          # (1, 3H)
#   i, o  = sigmoid(pre[:, :H]), sigmoid(pre[:, H:2H])
#   g     = tanh(pre[:, 2H:])
#   alpha = sigmoid(input_ @ aW_ih + a_bias + c_input @ aW_hh)   # (C, H)
#   w     = exp([i; alpha]); w /= w.sum(0)            # (C+1, H)
#   c1    = (([g; c_input]) * w).sum(0)               # (1, H)
#   h1    = o * tanh(c1)
#
# Strategy: tensor-parallel over the hidden (output-column) dim across 8
# cores (HS = 256 columns each); all post-matmul work is local to a shard
# so there are no collectives.  The kernel streams the weights through
# BOTH HWDGE rings (sync + scalar) in parallel, ~50/50 by bytes (each
# ring sustains ~215 GB/s when both are active; ~430 GB/s aggregate).
#
# Numerics: o-gate weights stay bf16 (h1 is directly sensitive); i/g and
# alpha weights are fp8 e4m3 pre-scaled by 256 (descaled in the ACT input
# scale) and consumed by DoubleRow pair-matmuls: a DR pair processes
# 2x256 moving columns in 256 cycles = 2 cols/cycle, halving PE time
# (measured 109 ns/pair at full clock).  The x-vector is cast to fp8
# on-chip (gpsimd) into a 16B-strided layout for the DR LDWEIGHTS
# pair-stride rule; the o matmuls use the bf16 x.  The softmax damps
# i/g/alpha quantization error by ~1/(C+1).
#
# Schedule notes (from perfetto traces):
#  - Each dma_start is a ~600 ns DIRECT2D on the issuing sequencer and a
#    ring only buffers a few transfers of descriptors, so later triggers
#    BLOCK the sequencer: scalar-ring triggers are interleaved with the
#    ACT compute; both rings LEAD with big transfers.
#  - The Tile scheduler orders per-engine by dependencies, so every
#    operand of an early op must itself land early (a late ones-vector
#    once stalled the bias matmuls that open the gate PSUM groups, which
#    serialized the whole back half).
#  - Global arrival order: alpha block -> i gate -> g gate -> o gate.
#    The alpha sigmoid/exp chain, the i-row exp(sigmoid) quadratic, the
#    (C+1)-row exp-normalize matmul, c1 and tanh(c1) all run under the
#    g/o streams; after the last o byte only 4 o matmuls, sigma(pre_o)
#    and the h1 product remain.
#  - A cold or starved PE runs at HALF clock and needs ~2-3 us of dense
#    matmuls to ramp: 10 dummy matmuls prime it, and dependency-anchored
#    "bridge" primes (reading freshly-landed weight tiles) re-feed the
#    activity monitor across the unavoidable early idle windows.
#  - Row transforms use the tanh table only and every ACT exp precedes
#    every tanh-family op (no ~1.3 us table reloads):
#      ew64 = exp(sigma(pre_i)) ~= sqrt(e)*(1 + t + t^2/2),
#             t = tanh(pre_i/2)/2  (max rel err ~1.4%, softmax-damped)
#      sigma(pre_o)*tanh(c1) via (1 + tanh(pre_o/2))/2.
#
import numpy as np

import concourse.bass as bass
import concourse.tile as tile
from concourse import bacc, mybir
from concourse.bass_utils import run_bass_kernel_spmd

NCORES = 8
H = 2048          # hidden size
IN = 2048         # input size
C = 64            # number of skip-word cell states
HS = H // NCORES  # hidden shard per core = 256
KG = IN + H       # gates contraction dim = 4096
KO_G = KG // 128  # 32 contraction k-slices for the gates matmuls
KO_A = IN // 128  # 16 contraction k-slices for the alpha matmuls
AWN = C + 2 * HS  # merged alpha-block column count = 576
F32 = mybir.dt.float32
F32R = mybir.dt.float32r
BF16 = mybir.dt.bfloat16
FP8 = mybir.dt.float8e4
DR = mybir.MatmulPerfMode.DoubleRow

ASCALE = 256.0    # fp8 pre-scale on the i/g/alpha weights (undone in ACT)
NPRIME = 10       # clock-gate priming matmuls (512 cols each)

_nc_cache = None


def _build_nc():
    """Build the single-core Bass program (same program runs on all 8 cores)."""
    nc = bacc.Bacc(
        "TRN2",
        target_bir_lowering=False,
        debug=False,
        enable_asserts=False,
        name="multi_input_lstm_cell",
    )

    # DRAM I/O (per-core shards; shapes identical on every core).  Weights
    # are host-pre-tiled to [ki=128, ko, n] so each chunk DMA reads long
    # contiguous per-partition segments at full HBM efficiency.
    xt = nc.dram_tensor("xt", [128, KO_G], BF16, kind="ExternalInput").ap()
    # merged alpha block: cols [ct | aW_ih shard | aW_hh shard]
    aw = nc.dram_tensor("aw", [128, KO_A, AWN], FP8, kind="ExternalInput").ap()
    # gates weights: i / g shards fp8, o shard bf16
    wgi = nc.dram_tensor("wgi", [128, KO_G, HS], FP8, kind="ExternalInput").ap()
    wgg = nc.dram_tensor("wgg", [128, KO_G, HS], FP8, kind="ExternalInput").ap()
    wgo = nc.dram_tensor("wgo", [128, KO_G, HS], BF16, kind="ExternalInput").ap()
    # bab[0, :] = [bias_i | bias_g | bias_o | alpha_bias] (i/g/alpha x256)
    bab = nc.dram_tensor("bab", [1, 4 * HS], F32, kind="ExternalInput").ap()
    cs = nc.dram_tensor("cs", [C, HS], F32R, kind="ExternalInput").ap()
    # all-ones [C+1, C]: column 0 feeds the ps01 reduction lhsT, row 0
    # feeds the alpha_wi broadcast lhsT
    ones = nc.dram_tensor("ones", [C + 1, C], F32R, kind="ExternalInput").ap()
    # hc[0, 0:256] = c1 shard, hc[0, 256:512] = h1 shard
    hc = nc.dram_tensor("hc", [1, 2 * HS], F32, kind="ExternalOutput").ap()

    with tile.TileContext(nc) as tc:
        _emit(tc, xt, aw, wgi, wgg, wgo, bab, cs, ones, hc)

    nc.compile()
    return nc


def _emit(tc, xt, aw, wgi, wgg, wgo, bab, cs, ones, hc):
    from contextlib import ExitStack

    nc = tc.nc
    EXP = mybir.ActivationFunctionType.Exp
    TANH = mybir.ActivationFunctionType.Tanh

    with ExitStack() as ctx:
        singles = ctx.enter_context(tc.tile_pool(name="singles", bufs=1))
        psum = ctx.enter_context(tc.tile_pool(name="psum", bufs=1, space="PSUM"))

        # ---- SBUF tiles -----------------------------------------------
        xt_t = singles.tile([128, KO_G], BF16, tag="xt")
        xt8_t = singles.tile([128, KO_G, 16], FP8, tag="xt8")
        bab_t = singles.tile([1, 4 * HS], F32, tag="bab")
        awA_t = singles.tile([128, KO_A // 2, AWN], FP8, tag="awA")
        awB_t = singles.tile([128, KO_A // 2, AWN], FP8, tag="awB")
        wgiA_t = singles.tile([128, KO_G // 2, HS], FP8, tag="wgiA")
        wgiB_t = singles.tile([128, KO_G // 2, HS], FP8, tag="wgiB")
        wggA_t = singles.tile([128, KO_G // 2, HS], FP8, tag="wggA")
        wggB_t = singles.tile([128, KO_G // 2, HS], FP8, tag="wggB")
        # o chunks: 12+4 k-slices per ring; small chunks last so the PE
        # tail after the last o byte is a few matmuls.
        O_CH = [(0, 12), (16, 12), (12, 4), (28, 4)]
        wo_ts = [singles.tile([128, sz, HS], BF16, tag=f"wo{i}", name=f"wo{i}")
                 for i, (_, sz) in enumerate(O_CH)]
        ones_t = singles.tile([C + 1, C], F32R, tag="ones_t")
        # emw: [exp-weights | merge*exp-weights], rows 0..63 = alpha rows,
        # row 64 = the i/g gate row.  cs lands in the merge half early.
        emw = singles.tile([C + 1, 2 * HS], F32R, tag="emw")

        # ---- sync ring -------------------------------------------------
        nc.sync.dma_start(out=bab_t[:], in_=bab)
        nc.sync.dma_start(out=awA_t[:], in_=aw[:, 0 : KO_A // 2, :])
        nc.sync.dma_start(out=wgiA_t[:], in_=wgi[:, 0 : KO_G // 2, :])
        nc.sync.dma_start(out=ones_t[:], in_=ones)
        nc.sync.dma_start(out=wggA_t[:], in_=wgg[:, 0 : KO_G // 2, :])
        nc.sync.dma_start(out=wo_ts[0][:], in_=wgo[:, 0:12, :])
        nc.sync.dma_start(out=wo_ts[2][:], in_=wgo[:, 12:16, :])

        # ---- scalar ring, first triggers -------------------------------
        # (the rest of the scalar triggers are interleaved with the ACT
        # compute below -- a trigger on a full ring blocks the sequencer)
        nc.scalar.dma_start(out=xt_t[:], in_=xt)
        nc.scalar.dma_start(out=awB_t[:], in_=aw[:, KO_A // 2 : KO_A, :])
        nc.scalar.dma_start(out=wgiB_t[:], in_=wgi[:, KO_G // 2 : KO_G, :])

        # Pre-warm the ACT exp table while everything else is idle.
        warm_t = singles.tile([1, 1], F32, tag="warm")
        nc.vector.memset(warm_t[:], 0.0)
        nc.scalar.activation(out=warm_t[:], in_=warm_t[:], func=EXP)

        # Priming fodder for the PE clock gate (contents irrelevant) and
        # the K=1 ones column for the bias rank-1 matmuls.
        prime_t = singles.tile([128, 512], BF16, tag="prime")
        nc.gpsimd.memset(prime_t[:], 0.0)
        one1_t = singles.tile([1, 1], F32, tag="one1")
        nc.vector.memset(one1_t[:], 1.0)

        # on-chip cast x (bf16) -> fp8, 16B-strided for DoubleRow LDWEIGHTS
        nc.gpsimd.tensor_scalar_add(out=xt8_t[:, :, 0:1], in0=xt_t[:],
                                    scalar1=0.0)

        # ---- PSUM tiles ------------------------------------------------
        pdum = psum.tile([1, 512], F32, tag="pdum")      # priming scratch
        pg_i = psum.tile([1, HS], F32, tag="pgi")        # pre_i
        pg_g = psum.tile([1, HS], F32, tag="pgg")        # pre_g
        pg_o = psum.tile([1, HS], F32, tag="pgo")        # pre_o
        pwi = psum.tile([1, HS], F32, tag="pwi")         # alpha_wi row
        pal = psum.tile([C, HS], F32, tag="pal")         # alpha pre-activation
        ps01 = psum.tile([1, 2 * HS], F32, tag="ps01")   # [sum ew | sum mg]

        # ---- PE: prime the clock gate with dense dummy matmuls ---------
        for _ in range(NPRIME):
            nc.tensor.matmul(pdum[:], lhsT=prime_t[:, 0:1], rhs=prime_t[:],
                             start=True, stop=True)

        # gate biases via K=1 rank-1 matmuls (open the gate PSUM groups)
        nc.tensor.matmul(pg_i[:], lhsT=one1_t[:], rhs=bab_t[:, 0:HS],
                         start=True, stop=False)
        nc.tensor.matmul(pg_g[:], lhsT=one1_t[:], rhs=bab_t[:, HS : 2 * HS],
                         start=True, stop=False)
        nc.tensor.matmul(pg_o[:], lhsT=one1_t[:], rhs=bab_t[:, 2 * HS : 3 * HS],
                         start=True, stop=False)

        # bridge primes: dense matmuls gated on awA so the PE activity
        # monitor stays fed across the wait for the alpha weights
        for r in range(3):
            nc.tensor.matmul(pdum[:], lhsT=prime_t[:, 0:1],
                             rhs=awA_t[:, r, 0:512],
                             start=True, stop=True)

        # ---- alpha matmuls (fp8 DoubleRow; pre-activations 256x scaled) --
        KH = KO_A // 2
        for half, aw_t in ((0, awA_t), (1, awB_t)):
            for kp in range(0, KH, 2):
                ko = half * KH + kp
                nc.tensor.matmul(pwi[:],
                                 lhsT=xt8_t[:, KO_A + ko : KO_A + ko + 2, 0:1],
                                 rhs=aw_t[:, kp : kp + 2, C : C + HS],
                                 start=(ko == 0), stop=(ko == KO_A - 2),
                                 perf_mode=DR)
            for kp in range(0, KH, 2):
                ko = half * KH + kp
                nc.tensor.matmul(pal[:], lhsT=aw_t[:, kp : kp + 2, 0:C],
                                 rhs=aw_t[:, kp : kp + 2, C + HS : C + 2 * HS],
                                 start=(ko == 0), stop=False,
                                 perf_mode=DR)
        wi_t = singles.tile([1, HS], F32R, tag="wi")
        nc.vector.tensor_add(out=wi_t[:], in0=pwi[:], in1=bab_t[:, 3 * HS : 4 * HS])
        nc.tensor.matmul(pal[:], lhsT=ones_t[0:1, 0:C], rhs=wi_t[:],
                         start=False, stop=True)

        # scalar-ring trigger: cs into the emw merge half (needed ~2 us
        # after the alpha exp chain below)
        nc.scalar.dma_start(out=emw[0:C, HS : 2 * HS], in_=cs)

        # alpha chain on ACT/DVE (under the i/g weight stream):
        # sigma via exp + fast reciprocal, then ew = exp(sigma), mg = cs * ew
        tmp_a = singles.tile([C, HS], F32, tag="tmp_a")
        nc.scalar.activation(out=tmp_a[:], in_=pal[:], func=EXP, scale=-1.0 / ASCALE)
        nc.vector.tensor_scalar_add(out=tmp_a[:], in0=tmp_a[:], scalar1=1.0)
        nc.vector.reciprocal_approx_fast(out=tmp_a[:], in_=tmp_a[:])
        nc.scalar.activation(out=emw[0:C, 0:HS], in_=tmp_a[:], func=EXP)
        nc.vector.tensor_mul(out=emw[0:C, HS : 2 * HS], in0=emw[0:C, HS : 2 * HS],
                             in1=emw[0:C, 0:HS])

        # scalar-ring trigger: g-gate B half
        nc.scalar.dma_start(out=wggB_t[:], in_=wgg[:, KO_G // 2 : KO_G, :])

        # bridge primes gated on the i-gate B-half arrival
        for r in range(2):
            nc.tensor.matmul(pdum[:, 0:256], lhsT=prime_t[:, 0:1],
                             rhs=wgiB_t[:, r, :],
                             start=True, stop=True)

        # ---- i then g gate matmuls (fp8 DoubleRow over k-slice pairs) --
        # A halves (sync ring) land before B halves (scalar ring).
        KHG = KO_G // 2
        for w_pair, pg in (((wgiA_t, wgiB_t), pg_i), ((wggA_t, wggB_t), pg_g)):
            for half, w_t in ((0, w_pair[0]), (1, w_pair[1])):
                for kp in range(0, KHG, 2):
                    ko = half * KHG + kp
                    nc.tensor.matmul(pg[:], lhsT=xt8_t[:, ko : ko + 2, 0:1],
                                     rhs=w_t[:, kp : kp + 2, :],
                                     start=False,
                                     stop=(half == 1 and kp == KHG - 2),
                                     perf_mode=DR)

        # scalar-ring triggers for the o chunks, interleaved with the
        # ACT tail ops
        nc.scalar.dma_start(out=wo_ts[1][:], in_=wgo[:, 16:28, :])

        # ---- i-row chain (ACT/DVE; under the g/o streams) --------------
        # Every ACT exp is done by now, so tanh-family ops are safe.
        SQE = 1.6487212707001282
        th_t = singles.tile([1, HS], F32, tag="th")
        nc.scalar.activation(out=th_t[:], in_=pg_i[:], func=TANH,
                             scale=0.5 / ASCALE)

        # g row: direct tanh
        nc.scalar.activation(out=emw[C : C + 1, HS : 2 * HS],
                             in_=pg_g[:], func=TANH, scale=1.0 / ASCALE)
        nc.scalar.dma_start(out=wo_ts[3][:], in_=wgo[:, 28:32, :])
        p_t = singles.tile([1, HS], F32, tag="p")
        nc.vector.tensor_scalar(out=p_t[:], in0=th_t[:],
                                scalar1=SQE / 8.0, scalar2=SQE / 2.0,
                                op0=mybir.AluOpType.mult,
                                op1=mybir.AluOpType.add)
        nc.vector.tensor_mul(out=p_t[:], in0=p_t[:], in1=th_t[:])
        nc.vector.tensor_scalar_add(out=emw[C : C + 1, 0:HS], in0=p_t[:],
                                    scalar1=SQE)
        nc.vector.tensor_mul(out=emw[C : C + 1, HS : 2 * HS],
                             in0=emw[C : C + 1, HS : 2 * HS],
                             in1=emw[C : C + 1, 0:HS])

        # ---- o gate matmuls, first big chunks (data long since landed) --
        def o_chunk(ci, last=False):
            kk, sz = O_CH[ci]
            for km in range(sz):
                nc.tensor.matmul(pg_o[:], lhsT=xt_t[:, kk + km : kk + km + 1],
                                 rhs=wo_ts[ci][:, km, :], start=False,
                                 stop=(last and km == sz - 1))

        o_chunk(0)
        o_chunk(1)

        # (C+1)-row exp-normalize reduction in one K=65 matmul
        nc.tensor.matmul(ps01[:], lhsT=ones_t[0 : C + 1, 0:1],
                         rhs=emw[0 : C + 1, :], start=True, stop=True)

        o_chunk(2)

        # c1 = s1 / s0 (s0 = sum of 65 exp values in [1, e]; the ~18-bit
        # fast reciprocal is plenty), then tanh(c1); all under the o stream.
        r_t = singles.tile([1, HS], F32, tag="r")
        nc.vector.reciprocal_approx_fast(out=r_t[:], in_=ps01[:, 0:HS])
        hc_t = singles.tile([1, 2 * HS], F32, tag="hc")
        nc.vector.tensor_mul(out=hc_t[:, 0:HS], in0=ps01[:, HS : 2 * HS], in1=r_t[:])
        t4_t = singles.tile([1, HS], F32, tag="t4")
        nc.scalar.activation(out=t4_t[:], in_=hc_t[:, 0:HS], func=TANH)

        o_chunk(3, last=True)

        # ---- final o tail: h1 = sigma(pre_o) * tanh(c1) ----------------
        # sigma(x)*t = (1 + tanh(x/2)) * (t/2): stays on the resident tanh
        # table (a Sigmoid op would trigger a ~1.3 us table load here).
        so_t = singles.tile([1, HS], F32, tag="so")
        nc.scalar.activation(out=so_t[:], in_=pg_o[:], func=TANH, scale=0.5)
        nc.vector.tensor_scalar(out=so_t[:], in0=so_t[:],
                                scalar1=0.5, scalar2=0.5,
                                op0=mybir.AluOpType.mult,
                                op1=mybir.AluOpType.add)
        nc.vector.tensor_mul(out=hc_t[:, HS : 2 * HS], in0=so_t[:], in1=t4_t[:])
        nc.sync.dma_start(out=hc, in_=hc_t[:])


def _shard_inputs(input_, c_input, h0, c0, weight_ih, weight_hh,
                  alpha_weight_ih, alpha_weight_hh, bias, alpha_bias):
    """Host-side scatter: column-shard the weights over the hidden dim.

    Weight matrices are cast to fp8/bf16 and pre-tiled to the [ki=128,
    ko, n] SBUF layout once for all cores; per-core shards are then
    cheap slices.
    """
    import ml_dtypes
    f32 = np.float32
    bf16 = ml_dtypes.bfloat16
    fp8 = ml_dtypes.float8_e4m3

    x_comb = np.concatenate([h0[0], input_[0]]).astype(f32)          # (4096,)
    xt = np.ascontiguousarray(x_comb.reshape(KO_G, 128).T).astype(bf16)
    # c_input.T tiled to [ki=128, ko=16, C]
    ct = np.ascontiguousarray(
        c_input.T.reshape(KO_A, 128, C).transpose(1, 0, 2)).astype(fp8)

    # gates weights: stack [W_hh; W_ih]; i/g cols scaled x256 to fp8,
    # o cols bf16; tile to [128, 32, n].
    wg_full = np.concatenate([weight_hh, weight_ih], axis=0).astype(f32)

    def ktile(a, dt):  # (4096, n) -> [128, 32, n]
        return np.ascontiguousarray(
            a.astype(dt).reshape(KO_G, 128, -1).transpose(1, 0, 2))

    wgi_t = ktile(wg_full[:, 0:H] * ASCALE, fp8)
    wgo_t = ktile(wg_full[:, H : 2 * H], bf16)
    wgg_t = ktile(wg_full[:, 2 * H : 3 * H] * ASCALE, fp8)
    del wg_full

    wai_t = np.ascontiguousarray(
        (alpha_weight_ih * ASCALE).astype(fp8).reshape(KO_A, 128, H).transpose(1, 0, 2))
    wah_t = np.ascontiguousarray(
        (alpha_weight_hh * ASCALE).astype(fp8).reshape(KO_A, 128, H).transpose(1, 0, 2))

    bias = np.asarray(bias, f32)
    alpha_bias = np.asarray(alpha_bias, f32)
    c_input = np.asarray(c_input, f32)

    in_maps = []
    for k in range(NCORES):
        cols = np.s_[k * HS : (k + 1) * HS]
        # merged alpha block [ct | aW_ih shard | aW_hh shard]
        aw = np.ascontiguousarray(
            np.concatenate([ct, wai_t[:, :, cols], wah_t[:, :, cols]], axis=2))
        bab = np.concatenate(
            [bias[0 * H + k * HS : 0 * H + (k + 1) * HS] * ASCALE,
             bias[2 * H + k * HS : 2 * H + (k + 1) * HS] * ASCALE,
             bias[1 * H + k * HS : 1 * H + (k + 1) * HS],
             alpha_bias[cols] * ASCALE])[None, :].astype(f32)
        in_maps.append({
            "xt": xt,
            "aw": aw,
            "wgi": np.ascontiguousarray(wgi_t[:, :, cols]),
            "wgg": np.ascontiguousarray(wgg_t[:, :, cols]),
            "wgo": np.ascontiguousarray(wgo_t[:, :, cols]),
            "bab": bab,
            "cs": np.ascontiguousarray(c_input[:, cols]),
            "ones": np.ones((C + 1, C), f32),
        })
    return in_maps


def _run(inputs, trace=False):
    global _nc_cache
    if _nc_cache is None:
        _nc_cache = _build_nc()
    nc = _nc_cache
    in_maps = _shard_inputs(**inputs)
    res = run_bass_kernel_spmd(nc, in_maps, core_ids=list(range(NCORES)), trace=trace)
    h1 = np.concatenate(
        [res.results[k]["hc"][:, HS : 2 * HS] for k in range(NCORES)], axis=1)
    c1 = np.concatenate(
        [res.results[k]["hc"][:, 0:HS] for k in range(NCORES)], axis=1)
    return (h1.astype(np.float32), c1.astype(np.float32)), res


def kernel(input_, c_input, h0, c0, weight_ih, weight_hh,
           alpha_weight_ih, alpha_weight_hh, bias, alpha_bias):
    inputs = dict(
        input_=np.asarray(input_, np.float32),
        c_input=np.asarray(c_input, np.float32),
        h0=np.asarray(h0, np.float32),
        c0=np.asarray(c0, np.float32),
        weight_ih=np.asarray(weight_ih, np.float32),
        weight_hh=np.asarray(weight_hh, np.float32),
        alpha_weight_ih=np.asarray(alpha_weight_ih, np.float32),
        alpha_weight_hh=np.asarray(alpha_weight_hh, np.float32),
        bias=np.asarray(bias, np.float32),
        alpha_bias=np.asarray(alpha_bias, np.float32),
    )
    out, _ = _run(inputs)
    return out


# revision 18
# speedup vs baseline: 1.1380x; 1.1380x over previous
# Self-contained Trainium2 Bass kernel for nn_MultiInputLSTMCell.
#
# Reference computation (all fp32):
#   pre   = h0 @ W_hh + bias + input_ @ W_ih          # (1, 3H)
#   i, o  = sigmoid(pre[:, :H]), sigmoid(pre[:, H:2H])
#   g     = tanh(pre[:, 2H:])
#   alpha = sigmoid(input_ @ aW_ih + a_bias + c_input @ aW_hh)   # (C, H)
#   w     = exp([i; alpha]); w /= w.sum(0)            # (C+1, H)
#   c1    = (([g; c_input]) * w).sum(0)               # (1, H)
#   h1    = o * tanh(c1)
#
# Strategy: tensor-parallel over the hidden (output-column) dim across 8
# cores (HS = 256 columns each); all post-matmul work is local to a shard
# so there are no collectives.  The kernel streams the weights through
# BOTH HWDGE rings (sync + scalar) in parallel, ~50/50 by bytes (each
# ring sustains ~215 GB/s when both are active; ~430 GB/s aggregate vs
# ~280 single-ring).
#
# Numerics: o-gate weights stay bf16 (h1 is directly sensitive); i/g and
# alpha weights are fp8 e4m3 pre-scaled by 256 (descaled in the ACT input
# scale) and consumed by DoubleRow pair-matmuls: a DR pair processes
# 2x256 moving columns in 256 cycles = 2 cols/cycle, halving PE time
# (measured 109 ns/pair at full clock).  The x-vector is cast to fp8
# on-chip (gpsimd) into a 16B-strided layout for the DR LDWEIGHTS
# pair-stride rule; the o matmuls use the bf16 x.  The softmax damps
# i/g/alpha quantization error by ~1/(C+1).
#
# Schedule notes (from perfetto traces):
#  - Each dma_start is a ~600 ns DIRECT2D on the issuing sequencer and a
#    ring only buffers a few transfers of descriptors, so later triggers
#    BLOCK the sequencer: the scalar ring's o-gate triggers are
#    interleaved with the ACT compute; both rings LEAD with big
#    transfers (a ring whose big data queues late starves ~2-3 us).
#  - The Tile scheduler orders per-engine by dependencies, so every
#    operand of an early op must itself land early (a late ones-vector
#    once stalled the bias matmuls that open the gate PSUM groups, which
#    serialized the whole back half).
#  - Global arrival order: alpha block -> i gate -> g gate -> o gate.
#    The alpha sigmoid/exp chain, the i-row exp(sigmoid) quadratic, the
#    (C+1)-row exp-normalize matmul, c1 and tanh(c1) all run under the
#    g/o streams; after the last o byte only 4 o matmuls, sigma(pre_o)
#    and the h1 product remain.
#  - A cold or starved PE runs at HALF clock and needs ~2-3 us of dense
#    matmuls to ramp: 10 dummy matmuls prime it, and dependency-anchored
#    "bridge" primes (reading freshly-landed weight tiles) re-feed the
#    activity monitor across the unavoidable early idle windows.
#  - Row transforms use the tanh table only and every ACT exp precedes
#    every tanh-family op (no ~1.3 us table reloads):
#      ew64 = exp(sigma(pre_i)) ~= sqrt(e)*(1 + t + t^2/2),
#             t = tanh(pre_i/2)/2  (max rel err ~1.4%, softmax-damped)
#      sigma(pre_o)*tanh(c1) via (1 + tanh(pre_o/2))/2.
#
import numpy as np

import concourse.bass as bass
import concourse.tile as tile
from concourse import bacc, mybir
from concourse.bass_utils import run_bass_kernel_spmd

NCORES = 8
H = 2048          # hidden size
IN = 2048         # input size
C = 64            # number of skip-word cell states
HS = H // NCORES  # hidden shard per core = 256
KG = IN + H       # gates contraction dim = 4096
KO_G = KG // 128  # 32 contraction k-slices for the gates matmuls
KO_A = IN // 128  # 16 contraction k-slices for the alpha matmuls
AWN = C + 2 * HS  # merged alpha-block column count = 576
F32 = mybir.dt.float32
F32R = mybir.dt.float32r
BF16 = mybir.dt.bfloat16
FP8 = mybir.dt.float8e4
DR = mybir.MatmulPerfMode.DoubleRow

ASCALE = 256.0    # fp8 pre-scale on the i/g/alpha weights (undone in ACT)
NPRIME = 10       # clock-gate priming matmuls (512 cols each)

_nc_cache = None


def _build_nc():
    """Build the single-core Bass program (same program runs on all 8 cores)."""
    nc = bacc.Bacc(
        "TRN2",
        target_bir_lowering=False,
        debug=False,
        enable_asserts=False,
        name="multi_input_lstm_cell",
    )

    # DRAM I/O (per-core shards; shapes identical on every core).  Weights
    # are host-pre-tiled to [ki=128, ko, n] so each chunk DMA reads long
    # contiguous per-partition segments at full HBM efficiency.
    xt = nc.dram_tensor("xt", [128, KO_G], BF16, kind="ExternalInput").ap()
    # merged alpha block: cols [ct | aW_ih shard | aW_hh shard]
    aw = nc.dram_tensor("aw", [128, KO_A, AWN], FP8, kind="ExternalInput").ap()
    # gates weights: i / g shards fp8, o shard bf16
    wgi = nc.dram_tensor("wgi", [128, KO_G, HS], FP8, kind="ExternalInput").ap()
    wgg = nc.dram_tensor("wgg", [128, KO_G, HS], FP8, kind="ExternalInput").ap()
    wgo = nc.dram_tensor("wgo", [128, KO_G, HS], BF16, kind="ExternalInput").ap()
    # bab[0, :] = [bias_i | bias_g | bias_o | alpha_bias] (i/g/alpha x256)
    bab = nc.dram_tensor("bab", [1, 4 * HS], F32, kind="ExternalInput").ap()
    cs = nc.dram_tensor("cs", [C, HS], F32R, kind="ExternalInput").ap()
    # all-ones [C+1, C]: column 0 feeds the ps01 reduction lhsT, row 0
    # feeds the alpha_wi broadcast lhsT
    ones = nc.dram_tensor("ones", [C + 1, C], F32R, kind="ExternalInput").ap()
    # hc[0, 0:256] = c1 shard, hc[0, 256:512] = h1 shard
    hc = nc.dram_tensor("hc", [1, 2 * HS], F32, kind="ExternalOutput").ap()

    with tile.TileContext(nc) as tc:
        _emit(tc, xt, aw, wgi, wgg, wgo, bab, cs, ones, hc)

    nc.compile()
    return nc


def _emit(tc, xt, aw, wgi, wgg, wgo, bab, cs, ones, hc):
    from contextlib import ExitStack

    nc = tc.nc
    EXP = mybir.ActivationFunctionType.Exp
    TANH = mybir.ActivationFunctionType.Tanh

    with ExitStack() as ctx:
        singles = ctx.enter_context(tc.tile_pool(name="singles", bufs=1))
        psum = ctx.enter_context(tc.tile_pool(name="psum", bufs=1, space="PSUM"))

        # ---- SBUF tiles -----------------------------------------------
        xt_t = singles.tile([128, KO_G], BF16, tag="xt")
        xt8_t = singles.tile([128, KO_G, 16], FP8, tag="xt8")
        bab_t = singles.tile([1, 4 * HS], F32, tag="bab")
        awA_t = singles.tile([128, KO_A // 2, AWN], FP8, tag="awA")
        awB_t = singles.tile([128, KO_A // 2, AWN], FP8, tag="awB")
        wgiA_t = singles.tile([128, KO_G // 2, HS], FP8, tag="wgiA")
        wgiB_t = singles.tile([128, KO_G // 2, HS], FP8, tag="wgiB")
        wggA_t = singles.tile([128, KO_G // 2, HS], FP8, tag="wggA")
        wggB_t = singles.tile([128, KO_G // 2, HS], FP8, tag="wggB")
        # o chunks: 8+4+4 k-slices per ring; small chunks last so the PE
        # tail after the last o byte is a few matmuls.
        O_CH = [(0, 8), (16, 8), (8, 4), (24, 4), (12, 4), (28, 4)]
        wo_ts = [singles.tile([128, sz, HS], BF16, tag=f"wo{i}", name=f"wo{i}")
                 for i, (_, sz) in enumerate(O_CH)]
        ones_t = singles.tile([C + 1, C], F32R, tag="ones_t")
        # emw: [exp-weights | merge*exp-weights], rows 0..63 = alpha rows,
        # row 64 = the i/g gate row.  cs lands in the merge half early.
        emw = singles.tile([C + 1, 2 * HS], F32R, tag="emw")

        # ---- sync ring -------------------------------------------------
        nc.sync.dma_start(out=bab_t[:], in_=bab)
        nc.sync.dma_start(out=awA_t[:], in_=aw[:, 0 : KO_A // 2, :])
        nc.sync.dma_start(out=wgiA_t[:], in_=wgi[:, 0 : KO_G // 2, :])
        nc.sync.dma_start(out=ones_t[:], in_=ones)
        nc.sync.dma_start(out=wggA_t[:], in_=wgg[:, 0 : KO_G // 2, :])
        nc.sync.dma_start(out=wo_ts[0][:], in_=wgo[:, 0:8, :])
        nc.sync.dma_start(out=wo_ts[2][:], in_=wgo[:, 8:12, :])
        nc.sync.dma_start(out=wo_ts[4][:], in_=wgo[:, 12:16, :])

        # ---- scalar ring, first triggers -------------------------------
        # (the rest of the scalar triggers are interleaved with the ACT
        # compute below -- a trigger on a full ring blocks the sequencer)
        nc.scalar.dma_start(out=awB_t[:], in_=aw[:, KO_A // 2 : KO_A, :])
        nc.scalar.dma_start(out=xt_t[:], in_=xt)
        nc.scalar.dma_start(out=wgiB_t[:], in_=wgi[:, KO_G // 2 : KO_G, :])

        # Pre-warm the ACT exp table while everything else is idle.
        warm_t = singles.tile([1, 1], F32, tag="warm")
        nc.vector.memset(warm_t[:], 0.0)
        nc.scalar.activation(out=warm_t[:], in_=warm_t[:], func=EXP)

        # Priming fodder for the PE clock gate (contents irrelevant) and
        # the K=1 ones column for the bias rank-1 matmuls.
        prime_t = singles.tile([128, 512], BF16, tag="prime")
        nc.gpsimd.memset(prime_t[:], 0.0)
        one1_t = singles.tile([1, 1], F32, tag="one1")
        nc.vector.memset(one1_t[:], 1.0)

        # on-chip cast x (bf16) -> fp8, 16B-strided for DoubleRow LDWEIGHTS
        nc.gpsimd.tensor_scalar_add(out=xt8_t[:, :, 0:1], in0=xt_t[:],
                                    scalar1=0.0)

        # ---- PSUM tiles ------------------------------------------------
        pdum = psum.tile([1, 512], F32, tag="pdum")      # priming scratch
        pg_i = psum.tile([1, HS], F32, tag="pgi")        # pre_i
        pg_g = psum.tile([1, HS], F32, tag="pgg")        # pre_g
        pg_o = psum.tile([1, HS], F32, tag="pgo")        # pre_o
        pwi = psum.tile([1, HS], F32, tag="pwi")         # alpha_wi row
        pal = psum.tile([C, HS], F32, tag="pal")         # alpha pre-activation
        ps01 = psum.tile([1, 2 * HS], F32, tag="ps01")   # [sum ew | sum mg]

        # ---- PE: prime the clock gate with dense dummy matmuls ---------
        for _ in range(NPRIME):
            nc.tensor.matmul(pdum[:], lhsT=prime_t[:, 0:1], rhs=prime_t[:],
                             start=True, stop=True)

        # gate biases via K=1 rank-1 matmuls (open the gate PSUM groups)
        nc.tensor.matmul(pg_i[:], lhsT=one1_t[:], rhs=bab_t[:, 0:HS],
                         start=True, stop=False)
        nc.tensor.matmul(pg_g[:], lhsT=one1_t[:], rhs=bab_t[:, HS : 2 * HS],
                         start=True, stop=False)
        nc.tensor.matmul(pg_o[:], lhsT=one1_t[:], rhs=bab_t[:, 2 * HS : 3 * HS],
                         start=True, stop=False)

        # bridge primes: dense matmuls gated on awA so the PE activity
        # monitor stays fed across the wait for the alpha weights
        for r in range(3):
            nc.tensor.matmul(pdum[:], lhsT=prime_t[:, 0:1],
                             rhs=awA_t[:, r, 0:512],
                             start=True, stop=True)

        # ---- alpha matmuls (fp8 DoubleRow; pre-activations 256x scaled) --
        KH = KO_A // 2
        for half, aw_t in ((0, awA_t), (1, awB_t)):
            for kp in range(0, KH, 2):
                ko = half * KH + kp
                nc.tensor.matmul(pwi[:],
                                 lhsT=xt8_t[:, KO_A + ko : KO_A + ko + 2, 0:1],
                                 rhs=aw_t[:, kp : kp + 2, C : C + HS],
                                 start=(ko == 0), stop=(ko == KO_A - 2),
                                 perf_mode=DR)
            for kp in range(0, KH, 2):
                ko = half * KH + kp
                nc.tensor.matmul(pal[:], lhsT=aw_t[:, kp : kp + 2, 0:C],
                                 rhs=aw_t[:, kp : kp + 2, C + HS : C + 2 * HS],
                                 start=(ko == 0), stop=False,
                                 perf_mode=DR)
        wi_t = singles.tile([1, HS], F32R, tag="wi")
        nc.vector.tensor_add(out=wi_t[:], in0=pwi[:], in1=bab_t[:, 3 * HS : 4 * HS])
        nc.tensor.matmul(pal[:], lhsT=ones_t[0:1, 0:C], rhs=wi_t[:],
                         start=False, stop=True)

        # scalar-ring trigger: cs into the emw merge half (needed ~2 us
        # after the alpha exp chain below)
        nc.scalar.dma_start(out=emw[0:C, HS : 2 * HS], in_=cs)

        # alpha chain on ACT/DVE (under the i/g weight stream):
        # sigma via exp + fast reciprocal, then ew = exp(sigma), mg = cs * ew
        tmp_a = singles.tile([C, HS], F32, tag="tmp_a")
        nc.scalar.activation(out=tmp_a[:], in_=pal[:], func=EXP, scale=-1.0 / ASCALE)
        nc.vector.tensor_scalar_add(out=tmp_a[:], in0=tmp_a[:], scalar1=1.0)
        nc.vector.reciprocal_approx_fast(out=tmp_a[:], in_=tmp_a[:])
        nc.scalar.activation(out=emw[0:C, 0:HS], in_=tmp_a[:], func=EXP)
        nc.vector.tensor_mul(out=emw[0:C, HS : 2 * HS], in0=emw[0:C, HS : 2 * HS],
                             in1=emw[0:C, 0:HS])

        # scalar-ring trigger: g-gate B half
        nc.scalar.dma_start(out=wggB_t[:], in_=wgg[:, KO_G // 2 : KO_G, :])

        # bridge primes gated on the i-gate B-half arrival
        for r in range(2):
            nc.tensor.matmul(pdum[:, 0:256], lhsT=prime_t[:, 0:1],
                             rhs=wgiB_t[:, r, :],
                             start=True, stop=True)

        # ---- i then g gate matmuls (fp8 DoubleRow over k-slice pairs) --
        # A halves (sync ring) land before B halves (scalar ring).
        KHG = KO_G // 2
        for w_pair, pg in (((wgiA_t, wgiB_t), pg_i), ((wggA_t, wggB_t), pg_g)):
            for half, w_t in ((0, w_pair[0]), (1, w_pair[1])):
                for kp in range(0, KHG, 2):
                    ko = half * KHG + kp
                    nc.tensor.matmul(pg[:], lhsT=xt8_t[:, ko : ko + 2, 0:1],
                                     rhs=w_t[:, kp : kp + 2, :],
                                     start=False,
                                     stop=(half == 1 and kp == KHG - 2),
                                     perf_mode=DR)

        # ---- i-row chain (ACT/DVE; under the g/o streams) --------------
        # Every ACT exp is done by now, so tanh-family ops are safe.
        SQE = 1.6487212707001282
        th_t = singles.tile([1, HS], F32, tag="th")
        nc.scalar.activation(out=th_t[:], in_=pg_i[:], func=TANH,
                             scale=0.5 / ASCALE)
        nc.scalar.dma_start(out=wo_ts[1][:], in_=wgo[:, 16:24, :])
        # g row: direct tanh
        nc.scalar.activation(out=emw[C : C + 1, HS : 2 * HS],
                             in_=pg_g[:], func=TANH, scale=1.0 / ASCALE)
        nc.scalar.dma_start(out=wo_ts[3][:], in_=wgo[:, 24:28, :])
        nc.scalar.dma_start(out=wo_ts[5][:], in_=wgo[:, 28:32, :])
        p_t = singles.tile([1, HS], F32, tag="p")
        nc.vector.tensor_scalar(out=p_t[:], in0=th_t[:],
                                scalar1=SQE / 8.0, scalar2=SQE / 2.0,
                                op0=mybir.AluOpType.mult,
                                op1=mybir.AluOpType.add)
        nc.vector.tensor_mul(out=p_t[:], in0=p_t[:], in1=th_t[:])
        nc.vector.tensor_scalar_add(out=emw[C : C + 1, 0:HS], in0=p_t[:],
                                    scalar1=SQE)
        nc.vector.tensor_mul(out=emw[C : C + 1, HS : 2 * HS],
                             in0=emw[C : C + 1, HS : 2 * HS],
                             in1=emw[C : C + 1, 0:HS])

        # ---- o gate matmuls, first big chunks (data long since landed) --
        def o_chunk(ci, last=False):
            kk, sz = O_CH[ci]
            for km in range(sz):
                nc.tensor.matmul(pg_o[:], lhsT=xt_t[:, kk + km : kk + km + 1],
                                 rhs=wo_ts[ci][:, km, :], start=False,
                                 stop=(last and km == sz - 1))

        o_chunk(0)
        o_chunk(1)

        # (C+1)-row exp-normalize reduction in one K=65 matmul
        nc.tensor.matmul(ps01[:], lhsT=ones_t[0 : C + 1, 0:1],
                         rhs=emw[0 : C + 1, :], start=True, stop=True)

        o_chunk(2)
        o_chunk(3)

        # c1 = s1 / s0 (s0 = sum of 65 exp values in [1, e]; the ~18-bit
        # fast reciprocal is plenty), then tanh(c1); all under the o stream.
        r_t = singles.tile([1, HS], F32, tag="r")
        nc.vector.reciprocal_approx_fast(out=r_t[:], in_=ps01[:, 0:HS])
        hc_t = singles.tile([1, 2 * HS], F32, tag="hc")
        nc.vector.tensor_mul(out=hc_t[:, 0:HS], in0=ps01[:, HS : 2 * HS], in1=r_t[:])
        t4_t = singles.tile([1, HS], F32, tag="t4")
        nc.scalar.activation(out=t4_t[:], in_=hc_t[:, 0:HS], func=TANH)

        o_chunk(4)
        o_chunk(5, last=True)

        # ---- final o tail: h1 = sigma(pre_o) * tanh(c1) ----------------
        # sigma(x)*t = (1 + tanh(x/2)) * (t/2): stays on the resident tanh
        # table (a Sigmoid op would trigger a ~1.3 us table load here).
        so_t = singles.tile([1, HS], F32, tag="so")
        nc.scalar.activation(out=so_t[:], in_=pg_o[:], func=TANH, scale=0.5)
        nc.vector.tensor_scalar(out=so_t[:], in0=so_t[:],
                                scalar1=0.5, scalar2=0.5,
                                op0=mybir.AluOpType.mult,
                                op1=mybir.AluOpType.add)
        nc.vector.tensor_mul(out=hc_t[:, HS : 2 * HS], in0=so_t[:], in1=t4_t[:])
        nc.sync.dma_start(out=hc, in_=hc_t[:])


def _shard_inputs(input_, c_input, h0, c0, weight_ih, weight_hh,
                  alpha_weight_ih, alpha_weight_hh, bias, alpha_bias):
    """Host-side scatter: column-shard the weights over the hidden dim.

    Weight matrices are cast to fp8/bf16 and pre-tiled to the [ki=128,
    ko, n] SBUF layout once for all cores; per-core shards are then
    cheap slices.
    """
    import ml_dtypes
    f32 = np.float32
    bf16 = ml_dtypes.bfloat16
    fp8 = ml_dtypes.float8_e4m3

    x_comb = np.concatenate([h0[0], input_[0]]).astype(f32)          # (4096,)
    xt = np.ascontiguousarray(x_comb.reshape(KO_G, 128).T).astype(bf16)
    # c_input.T tiled to [ki=128, ko=16, C]
    ct = np.ascontiguousarray(
        c_input.T.reshape(KO_A, 128, C).transpose(1, 0, 2)).astype(fp8)

    # gates weights: stack [W_hh; W_ih]; i/g cols scaled x256 to fp8,
    # o cols bf16; tile to [128, 32, n].
    wg_full = np.concatenate([weight_hh, weight_ih], axis=0).astype(f32)

    def ktile(a, dt):  # (4096, n) -> [128, 32, n]
        return np.ascontiguousarray(
            a.astype(dt).reshape(KO_G, 128, -1).transpose(1, 0, 2))

    wgi_t = ktile(wg_full[:, 0:H] * ASCALE, fp8)
    wgo_t = ktile(wg_full[:, H : 2 * H], bf16)
    wgg_t = ktile(wg_full[:, 2 * H : 3 * H] * ASCALE, fp8)
    del wg_full

    wai_t = np.ascontiguousarray(
        (alpha_weight_ih * ASCALE).astype(fp8).reshape(KO_A, 128, H).transpose(1, 0, 2))
    wah_t = np.ascontiguousarray(
        (alpha_weight_hh * ASCALE).astype(fp8).reshape(KO_A, 128, H).transpose(1, 0, 2))

    bias = np.asarray(bias, f32)
    alpha_bias = np.asarray(alpha_bias, f32)
    c_input = np.asarray(c_input, f32)

    in_maps = []
    for k in range(NCORES):
        cols = np.s_[k * HS : (k + 1) * HS]
        # merged alpha block [ct | aW_ih shard | aW_hh shard]
        aw = np.ascontiguousarray(
            np.concatenate([ct, wai_t[:, :, cols], wah_t[:, :, cols]], axis=2))
        bab = np.concatenate(
            [bias[0 * H + k * HS : 0 * H + (k + 1) * HS] * ASCALE,
             bias[2 * H + k * HS : 2 * H + (k + 1) * HS] * ASCALE,
             bias[1 * H + k * HS : 1 * H + (k + 1) * HS],
             alpha_bias[cols] * ASCALE])[None, :].astype(f32)
        in_maps.append({
            "xt": xt,
            "aw": aw,
            "wgi": np.ascontiguousarray(wgi_t[:, :, cols]),
            "wgg": np.ascontiguousarray(wgg_t[:, :, cols]),
            "wgo": np.ascontiguousarray(wgo_t[:, :, cols]),
            "bab": bab,
            "cs": np.ascontiguousarray(c_input[:, cols]),
            "ones": np.ones((C + 1, C), f32),
        })
    return in_maps


def _run(inputs, trace=False):
    global _nc_cache
    if _nc_cache is None:
        _nc_cache = _build_nc()
    nc = _nc_cache
    in_maps = _shard_inputs(**inputs)
    res = run_bass_kernel_spmd(nc, in_maps, core_ids=list(range(NCORES)), trace=trace)
    h1 = np.concatenate(
        [res.results[k]["hc"][:, HS : 2 * HS] for k in range(NCORES)], axis=1)
    c1 = np.concatenate(
        [res.results[k]["hc"][:, 0:HS] for k in range(NCORES)], axis=1)
    return (h1.astype(np.float32), c1.astype(np.float32)), res


def kernel(input_, c_input, h0, c0, weight_ih, weight_hh,
           alpha_weight_ih, alpha_weight_hh, bias, alpha_bias):
    inputs = dict(
        input_=np.asarray(input_, np.float32),
        c_input=np.asarray(c_input, np.float32),
        h0=np.asarray(h0, np.float32),
        c0=np.asarray(c0, np.float32),
        weight_ih=np.asarray(weight_ih, np.float32),
        weight_hh=np.asarray(weight_hh, np.float32),
        alpha_weight_ih=np.asarray(alpha_weight_ih, np.float32),
        alpha_weight_hh=np.asarray(alpha_weight_hh, np.float32),
        bias=np.asarray(bias, np.float32),
        alpha_bias=np.asarray(alpha_bias, np.float32),
    )
    out, _ = _run(inputs)
    return out
